# revision 7
# baseline (speedup 1.0000x reference)
"""Trainium2 Bass kernel for a cross-attention layer (v3, deferred-K/V).

Reference computation (per batch b):
    vision = inputs[b, :, :1024]; text = inputs[b, :, 1024:]
    Q = vision @ Wq.T + bq;  K = text @ Wk.T + bk;  V = text @ Wv.T + bv
    attn = softmax(Q @ K.T / 32, axis=-1)                 # [S, S]
    cav  = attn @ V                                       # [S, 1024]
    cat  = attn.T @ vision                                # [S, 1024]

Restructure (exact up to float assoc; no collectives needed):
    softmax is invariant to per-row constants, so with
      A = Wq.T @ Wk   [d, e]   (host precompute, tiny)
      w = Wk.T @ bq   [e]
    scores ~ (vision @ A + w) @ text.T        (drops per-row consts only)
    and since softmax rows sum to 1:
      cav = (attn @ text) @ Wv.T + bv
    so the K and V projections (and the core-pair K/V exchanges of v2)
    disappear entirely.

Sharding: 8 cores = 4 batches x 2 query-halves (1024 q rows each).
Each core holds its own vision half (two layouts) plus the FULL text of
its batch (two layouts: e-major for scores, k-major for attn@text).

Per-core algorithm (all SBUF-resident, no DRAM spill):
  0:  Q'T[e,q] = A.T @ visionT + w          (lhsT=A[d,e], rhs=visT[d,q])
  1:  per q-tile qt (128 rows):
      a: exp_s[qt, 0:2048] = exp((Q'T.T @ txtT) / 32), Z via ACT accum
      b: cav(qt-1) = (M1T(qt-1).T @ WvT) * 1/Z + bv  -> DMA  (pipelined
         one qt behind so PSUM WARs hide under other PE work)
      c: PE-transpose exp_s[qt] 128x128 blocks into one PSUM tile ->
         one DVE copy -> expT scratch [k,q]
      d: M1T[d, qt] = txtK.T @ expT (full 2048-key contraction; this
         qt's 128 q-cols are complete because scores span all keys)
      e: vis_sc[qt] = vision[qt] * 1/Z (in place)
  2:  cat[k,:] = sum_qt exp_s[qt,k].T @ vis_sc[qt] -> DMA (partial; host
      sums the core pair)

PSUM (8 banks): big 2x[128,1024] (Q', scores, cat; 4 banks) +
mc 1x[128,1024] (M1T psum then cav psum; 2 banks) + tr 1x[128,16,128]
bf16 (all 16 transposes of a qt; 2 banks).
SBUF peak ~185KB/partition: dat (txtT 32K, txtK 32K, wv 16K, vis 16K),
qt_sb 16K, exps 32K, scratches ~30K; the phase-0 pool (A 16K, vT 16K)
closes before the attn pool opens.
"""

import numpy as np
import ml_dtypes

B, S, D = 4, 2048, 1024
QH = 1024          # query rows per core
NCORES = 8

_CACHE = {}


def _build(reps=1):
    import contextlib

    import concourse.mybir as mybir
    from concourse import bacc
    from concourse.masks import make_identity
    from concourse.tile import TileContext

    DT = mybir.dt.bfloat16
    F32 = mybir.dt.float32
    AF = mybir.ActivationFunctionType
    ADD = mybir.AluOpType.add
    SCALE = float(1.0 / np.sqrt(np.float32(D)))

    nc = bacc.Bacc()
    visionT = nc.dram_tensor("visionT", [D, QH], DT, kind="ExternalInput")
    vision = nc.dram_tensor("vision", [QH, D], DT, kind="ExternalInput")
    txtT = nc.dram_tensor("txtT", [D, S], DT, kind="ExternalInput")
    txtK = nc.dram_tensor("txtK", [S, D], DT, kind="ExternalInput")
    amat = nc.dram_tensor("amat", [D, D], DT, kind="ExternalInput")
    wvT = nc.dram_tensor("wvT", [D, D], DT, kind="ExternalInput")
    wp = nc.dram_tensor("wp", [128, 8], F32, kind="ExternalInput")
    bvb = nc.dram_tensor("bvb", [1, D], F32, kind="ExternalInput")
    cav_o = nc.dram_tensor("cav", [QH, D], F32, kind="ExternalOutput")
    cat_o = nc.dram_tensor("catp", [S, D], F32, kind="ExternalOutput")

    visionT_r = visionT.rearrange("(dt p) q -> p dt q", p=128)
    vision_r = vision.rearrange("(qt p) d -> p qt d", p=128)
    txtT_r = txtT.rearrange("(et p) k -> p et k", p=128)
    txtK_r = txtK.rearrange("(kt p) d -> p kt d", p=128)
    amat_r = amat.rearrange("(dt p) e -> p dt e", p=128)
    wv_r = wvT.rearrange("(dt p) e -> p dt e", p=128)
    cav_r = cav_o.rearrange("(qt p) e -> p qt e", p=128)
    cat_r = cat_o.rearrange("(kt p) d -> p kt d", p=128)

    with TileContext(nc) as tc:
        rep_cm = tc.For_i(0, reps, 1) if reps > 1 else contextlib.nullcontext()
        with (
            rep_cm,
            tc.tile_pool(name="const", bufs=1) as const,
            tc.tile_pool(name="dat", bufs=1) as dat,
            tc.tile_pool(name="stats", bufs=1) as stats,
            tc.tile_pool(name="bigps", bufs=1, space="PSUM") as bigps,
            tc.tile_pool(name="auxps", bufs=1, space="PSUM") as auxps,
            tc.tile_pool(name="mcps", bufs=2, space="PSUM") as mcps,
        ):
            wp_sb = const.tile([128, 8], F32)
            bv_bc = const.tile([128, D], F32)
            ident = const.tile([128, 128], DT)
            qt_sb = const.tile([128, 8, QH], DT)

            txtT_sb = dat.tile([128, 8, S], DT)
            txtK_sb = dat.tile([128, 16, D], DT)
            wv_sb = dat.tile([128, 8, D], DT)
            vis_sb = dat.tile([128, 8, D], DT)

            z_own = stats.tile([128, 8], F32)
            z_acc = stats.tile([128, 8], F32)
            invz = stats.tile([128, 8], F32)

            # PE warm-up: throwaway matmuls during the initial input-DMA
            # window so the HAM clock gate un-throttles (1.2 -> 2.4 GHz)
            # before the first real matmul.
            warm = const.tile([128, 512], DT)
            nc.vector.memset(warm, 1.0)
            for w in range(32):
                wps = bigps.tile([128, 2048], F32, tag="big", name=f"warm{w}")
                nc.tensor.matmul(wps[:, 0:512], lhsT=warm[:, 0:128],
                                 rhs=warm, start=True, stop=True)

            # ---- input loads + phase 0 (A/visT freed after) ----
            with tc.tile_pool(name="inp", bufs=1) as inp:
                # startup-critical loads first, in per-dt chunks so the
                # first Q' matmuls gate on 512KB, not 4MB.
                a_sb = inp.tile([128, 8, D], DT)
                for dt in range(4):
                    nc.sync.dma_start(out=a_sb[:, 2 * dt:2 * dt + 2, :],
                                      in_=amat_r[:, 2 * dt:2 * dt + 2, :])
                vT_sb = inp.tile([128, 8, QH], DT)
                for dt in range(4):
                    nc.sync.dma_start(out=vT_sb[:, 2 * dt:2 * dt + 2, :],
                                      in_=visionT_r[:, 2 * dt:2 * dt + 2, :])
                nc.sync.dma_start(out=wp_sb, in_=wp[:])
                make_identity(nc, ident)
                # big streaming loads, ordered by first use
                for et in range(4):
                    nc.sync.dma_start(
                        out=txtT_sb[:, 2 * et:2 * et + 2, :],
                        in_=txtT_r[:, 2 * et:2 * et + 2, :])
                for kt in range(4):
                    nc.sync.dma_start(
                        out=txtK_sb[:, 4 * kt:4 * kt + 4, :],
                        in_=txtK_r[:, 4 * kt:4 * kt + 4, :])
                nc.sync.dma_start(out=wv_sb, in_=wv_r)
                nc.sync.dma_start(out=vis_sb, in_=vision_r)
                nc.sync.dma_start(out=bv_bc, in_=bvb[:].to_broadcast((128, D)))

                # Phase 0: Q'T[e,q] = A.T @ visionT, + w as per-partition
                # bias. Two et-groups per [128,2048] PSUM tile.
                for ep in range(4):
                    ps = bigps.tile([128, 2048], F32, tag="big")
                    for sub in range(2):
                        et = 2 * ep + sub
                        for qc in range(2):
                            for dt in range(8):
                                nc.tensor.matmul(
                                    ps[:, sub * 1024 + qc * 512:
                                       sub * 1024 + (qc + 1) * 512],
                                    lhsT=a_sb[:, dt, et * 128:(et + 1) * 128],
                                    rhs=vT_sb[:, dt, qc * 512:(qc + 1) * 512],
                                    start=(dt == 0),
                                    stop=(dt == 7),
                                )
                        nc.scalar.activation(
                            out=qt_sb[:, et, :],
                            in_=ps[:, sub * 1024:(sub + 1) * 1024],
                            func=AF.Identity,
                            bias=wp_sb[:, et:et + 1],
                            scale=1.0,
                        )

            # ---- Phases 1/2: attn pool reuses the inp region ----
            with tc.tile_pool(name="attn", bufs=1) as attn:
                exps = attn.tile([128, 8, S], DT, tag="exps")
                m1_tiles = {}

                def cav_qt(qt):
                    """cav[qt] = (M1T[qt].T @ WvT) * invz + bv -> DMA."""
                    psc = auxps.tile([128, 1024], F32, tag="aux")
                    m1_sb = m1_tiles[qt // 4]
                    qo = (qt % 4) * 128
                    for ec in range(2):
                        for dt in range(8):
                            nc.tensor.matmul(
                                psc[:, ec * 512:(ec + 1) * 512],
                                lhsT=m1_sb[:, dt, qo:qo + 128],
                                rhs=wv_sb[:, dt, ec * 512:(ec + 1) * 512],
                                start=(dt == 0),
                                stop=(dt == 7),
                            )
                    cav_n = attn.tile([128, D], F32, tag="cavn", bufs=2)
                    nc.scalar.activation(
                        out=cav_n,
                        in_=psc,
                        func=AF.Copy,
                        scale=invz[:, qt:qt + 1],
                    )
                    cav_st = attn.tile([128, D], F32, tag="cavo", bufs=2)
                    nc.vector.tensor_tensor(
                        out=cav_st, in0=cav_n, in1=bv_bc, op=ADD)
                    nc.sync.dma_start(out=cav_r[:, qt, :], in_=cav_st)

                for blk in range(2):
                    et_scr = attn.tile([128, 16, 512], DT, tag="etr")
                    for qi in range(4):
                        qt = blk * 4 + qi
                        # a: scores + exp + Z, both key halves in one tile
                        ps = bigps.tile([128, 2048], F32, tag="big")
                        for h in range(2):
                            for kc in range(2):
                                for et in range(8):
                                    nc.tensor.matmul(
                                        ps[:, h * 1024 + kc * 512:
                                           h * 1024 + (kc + 1) * 512],
                                        lhsT=qt_sb[:, et,
                                                   qt * 128:(qt + 1) * 128],
                                        rhs=txtT_sb[:, et,
                                                    h * 1024 + kc * 512:
                                                    h * 1024 + (kc + 1) * 512],
                                        start=(et == 0),
                                        stop=(et == 7),
                                    )
                            zp = attn.tile([128, 1], F32, tag="zp", bufs=4)
                            nc.scalar.activation(
                                out=exps[:, qt, h * 1024:(h + 1) * 1024],
                                in_=ps[:, h * 1024:(h + 1) * 1024],
                                func=AF.Exp,
                                scale=SCALE,
                                accum_out=zp,
                            )
                            if h == 0:
                                nc.vector.tensor_copy(
                                    out=z_own[:, qt:qt + 1], in_=zp)
                            else:
                                nc.vector.tensor_add(
                                    out=z_acc[:, qt:qt + 1],
                                    in0=z_own[:, qt:qt + 1],
                                    in1=zp,
                                )
                        nc.vector.reciprocal(
                            out=invz[:, qt:qt + 1], in_=z_acc[:, qt:qt + 1])

                        # b: transpose this qt's exp row-block via normal
                        # matmul against identity (runs at matmul speed,
                        # not transpose-mode speed), 2 rounds of 8.
                        for half in range(2):
                            pst = auxps.tile([128, 1024], F32, tag="aux")
                            for ks in range(8):
                                kst = half * 8 + ks
                                nc.tensor.matmul(
                                    pst[:, ks * 128:(ks + 1) * 128],
                                    lhsT=exps[:, qt,
                                              kst * 128:(kst + 1) * 128],
                                    rhs=ident,
                                    start=True,
                                    stop=True,
                                )
                            nc.vector.tensor_copy(
                                out=et_scr[:, half * 8:(half + 1) * 8,
                                           qi * 128:(qi + 1) * 128],
                                in_=pst,
                            )

                        # c: previous block's cav, one per qt (spreads the
                        # PSUM WARs and the output DMA)
                        if blk == 1:
                            cav_qt(qi)

                        # d: vis_sc = vision * 1/Z (in place)
                        nc.scalar.activation(
                            out=vis_sb[:, qt, :],
                            in_=vis_sb[:, qt, :],
                            func=AF.Copy,
                            scale=invz[:, qt:qt + 1],
                        )

                    # e: M1T[d, blk-cols] = txtK.T @ expT at N=512
                    m1_sb = attn.tile([128, 8, 512], DT, tag="m1", bufs=2)
                    for dt in range(8):
                        psm = mcps.tile([128, 512], F32, tag="mc")
                        for kt in range(16):
                            nc.tensor.matmul(
                                psm,
                                lhsT=txtK_sb[:, kt, dt * 128:(dt + 1) * 128],
                                rhs=et_scr[:, kt, :],
                                start=(kt == 0),
                                stop=(kt == 15),
                            )
                        nc.vector.tensor_copy(out=m1_sb[:, dt, :], in_=psm)
                    m1_tiles[blk] = m1_sb

                # 2: cat[k,:] = sum_q exp_s[q,k] vis_sc[q,:], kk-pairs per
                # PSUM tile; block-1 cavs interleaved with the first pairs.
                for kp in range(8):
                    if kp < 4:
                        cav_qt(4 + kp)
                    ps = bigps.tile([128, 2048], F32, tag="big")
                    for sub in range(2):
                        kk = 2 * kp + sub
                        for dc in range(2):
                            for qt in range(8):
                                nc.tensor.matmul(
                                    ps[:, sub * 1024 + dc * 512:
                                       sub * 1024 + (dc + 1) * 512],
                                    lhsT=exps[:, qt, kk * 128:(kk + 1) * 128],
                                    rhs=vis_sb[:, qt, dc * 512:(dc + 1) * 512],
                                    start=(qt == 0),
                                    stop=(qt == 7),
                                )
                        cat_sb = attn.tile([128, D], F32, tag="cato", bufs=2)
                        nc.vector.tensor_copy(
                            out=cat_sb, in_=ps[:, sub * 1024:(sub + 1) * 1024])
                        nc.sync.dma_start(out=cat_r[:, kk, :], in_=cat_sb)
    nc.compile()
    return nc


def _get_nc(reps=1):
    key = ("nc", reps)
    if key not in _CACHE:
        _CACHE[key] = _build(reps)
    return _CACHE[key]


def _prep_in_maps(inputs, Wq, bq, Wk, bk, Wv, bv):
    bf = ml_dtypes.bfloat16
    x = np.asarray(inputs, np.float32)
    Wq32 = np.asarray(Wq, np.float32)
    Wk32 = np.asarray(Wk, np.float32)
    Wv32 = np.asarray(Wv, np.float32)
    bq32 = np.asarray(bq, np.float32)
    # host weight preprocessing (tiny): A = Wq.T @ Wk, w = Wk.T @ bq
    amat = np.ascontiguousarray((Wq32.T @ Wk32).astype(bf))
    w = Wk32.T @ bq32
    wp = np.ascontiguousarray(w.reshape(8, 128).T.astype(np.float32))
    wvT = np.ascontiguousarray(Wv32.T.astype(bf))
    bvb = np.asarray(bv, np.float32).reshape(1, D)
    txtTs, txtKs = [], []
    for b in range(B):
        txt = x[b, :, D:]
        txtKs.append(np.ascontiguousarray(txt.astype(bf)))
        txtTs.append(np.ascontiguousarray(txt.T.astype(bf)))
    in_maps = []
    for c in range(NCORES):
        b, h = divmod(c, 2)
        visc = x[b, h * QH:(h + 1) * QH, :D]
        in_maps.append({
            "visionT": np.ascontiguousarray(visc.T.astype(bf)),
            "vision": np.ascontiguousarray(visc.astype(bf)),
            "txtT": txtTs[b], "txtK": txtKs[b],
            "amat": amat, "wvT": wvT, "wp": wp, "bvb": bvb,
        })
    return in_maps


def run_on_device(in_maps, trace=False, reps=1):
    from concourse.bass_utils import run_bass_kernel_spmd

    nc = _get_nc(reps)
    return run_bass_kernel_spmd(
        nc, in_maps, core_ids=list(range(NCORES)), trace=trace
    )


def _gather(results):
    cav_full = np.empty((B, S, D), np.float32)
    cat_full = np.zeros((B, S, D), np.float32)
    for c in range(NCORES):
        b, h = divmod(c, 2)
        cav_full[b, h * QH:(h + 1) * QH] = results[c]["cav"]
        cat_full[b] += results[c]["catp"]
    return cav_full, cat_full


def kernel(**inputs):
    in_maps = _prep_in_maps(**inputs)
    last_err = None
    for _ in range(3):  # transient axon/NRT hiccups happen
        try:
            res = run_on_device(in_maps, trace=False)
            return _gather(res.results)
        except Exception as e:
            last_err = e
    raise last_err
